# revision 54
# baseline (speedup 1.0000x reference)
"""CS-TreeLSTM (BRANCH=16, DEPTH=4, IN=HID=512) on 8 Trainium2 NeuronCores.

Strategy (data-parallel over subtrees, per the sharding hint):
  - Each core owns 8192 leaves and 512 level-3 nodes; levels 2..0 run on the
    host in float64 from the per-core level-3 outputs (i3/u3/o3/fcsum3),
    removing the serial small-matmul device tail entirely.
  - Activations live transposed on-chip: [hid/in on partitions, nodes on free].
  - Gate matmuls run as f32r (fp32-reduced, ~tf32) at bf16 PE speed.
  - Sibling sums (h_sum, sum_k f_k*C_k) are grouped free-dim reduces on DVE.
  - The parent-x + bias term of the forget gate is pre-written into PSUM by
    the (otherwise idle) Pool engine as a broadcast copy of fx3T; the Wfh
    matmuls then accumulate on top (start=False), replacing the indicator
    aug-matmul (which cost a full 512-row PE pass per m-tile).
  - Gates run i, u, o per chunk so C=i*u is ready while o still streams,
    hiding the f-path and hsum latency of the following consumers.
  - Input DMAs are spread across the SP, Pool and DVE queues: descriptor
    generation (~0.6us per DMA) serializes per queue and was the startup
    critical path.

Built on bacc.Bacc so multi-semaphore waits are legalized into event
semaphores automatically (TRN2 allows one sync wait per instruction).
"""

import sys

sys.path.insert(0, "/opt/trn_rl_repo")

import numpy as np

BRANCH = 16
DEPTH = 4
IN = 512
HID = 512
NC_N = 8
SIZES = [BRANCH**d for d in range(DEPTH + 1)]  # [1,16,256,4096,65536]
OFFS = [0, 1, 17, 273, 4369, 69905]
XT_COLS = 8192 + 512  # leaf x + level-3 x
C3_OFF = 8192
N_CHUNK = 16

_CACHE = {}


def _build_nc(cfg=None):
    cfg = cfg or {}
    from concourse import bacc
    import concourse.mybir as mybir
    import concourse.tile as tile

    F32 = mybir.dt.float32
    F32R = mybir.dt.float32r
    ACTF = mybir.ActivationFunctionType
    AX = mybir.AxisListType
    OP = mybir.AluOpType

    nc = bacc.Bacc()

    xt = nc.declare_dram_parameter("xt", [IN, XT_COLS], F32, isOutput=False)
    wname = ["wix", "wih", "wox", "woh", "wux", "wuh", "wfx", "wfh"]
    wps = {n: nc.declare_dram_parameter(n, [IN, HID], F32, isOutput=False) for n in wname}
    bT = {g: nc.declare_dram_parameter("bT" + g, [128, 4], F32, isOutput=False) for g in "iouf"}
    out_t = {
        n: nc.declare_dram_parameter(n, [128, 4, 512], F32, isOutput=True)
        for n in ("i3T", "u3T", "o3T", "fc3T")
    }

    def t_view(h):  # DRAM [512, n] -> [128 part, 4 ktile, n] view
        return h[:, :].rearrange("(t p) n -> p t n", p=128)

    from contextlib import ExitStack

    with tile.TileContext(nc) as tc, ExitStack() as ctx:
        consts = ctx.enter_context(tc.tile_pool(name="consts", bufs=1))
        stream = ctx.enter_context(tc.tile_pool(name="stream", bufs=cfg.get("stream", 5)))
        workA = ctx.enter_context(tc.tile_pool(name="workA", bufs=cfg.get("workA", 2)))
        workB = ctx.enter_context(tc.tile_pool(name="workB", bufs=cfg.get("workB", 2)))
        longp = ctx.enter_context(tc.tile_pool(name="longp", bufs=1))
        psum = ctx.enter_context(tc.tile_pool(name="psum", bufs=cfg.get("psum", 8), space="PSUM"))

        # ---------------- constants / weights ----------------
        # Startup critical path: descriptor generation serializes per DMA
        # queue, so the first chunk's x (SP queue), wix (split between Pool
        # and DVE queues) and wux (Pool) are spread to be ready just in time
        # for the i,u,o gate ladder of chunk 0. The h-part weights ride the
        # leaf x stream pool late.
        W = {}
        bTs = {}
        stream_tiles = {}

        def load_w(n, eng=None):
            W[n] = consts.tile([128, 4, HID], F32R, tag="w_" + n, name="w_" + n)
            (eng or nc.sync).dma_start(out=W[n][:, :, :], in_=t_view(wps[n]).bitcast(F32R))

        def load_chunk(c, eng=None):
            t = stream.tile([128, 4, 512], F32R, tag="xt_c", name=f"xt_c{c}")
            (eng or nc.sync).dma_start(
                out=t[:, :, :], in_=t_view(xt)[:, :, c * 512 : (c + 1) * 512].bitcast(F32R)
            )
            stream_tiles[c] = t
            return t

        # DMA transfers serialize in one FIFO ordered by descriptor-gen
        # completion, and SP/ACT queues share the (serial) HWDGE unit; only
        # Pool's SWDGE generates in parallel. So: x chunks and the big
        # weights go on SP in exact consumption order (i0,i1,u0,u1,o0,o1,
        # fx3T,f0 of the paired c0/c1 warmup below), while wix + biases ride
        # the Pool lane whose transfers interleave with SP's early pieces.
        t0 = stream.tile([128, 4, 512], F32R, tag="xt_c", name="xt_c0")
        stream_tiles[0] = t0
        W["wix"] = consts.tile([128, 4, HID], F32R, tag="w_wix", name="w_wix")
        for k in range(4):
            nc.sync.dma_start(out=t0[:, k, :], in_=t_view(xt)[:, k, 0:512].bitcast(F32R))
            nc.gpsimd.dma_start(out=W["wix"][:, k, :], in_=t_view(wps["wix"])[:, k, :].bitcast(F32R))
        for g in "iouf":
            bTs[g] = consts.tile([128, 4], F32, tag="bT" + g, name="bT" + g)
            nc.gpsimd.dma_start(out=bTs[g][:, :], in_=bT[g][:, :])

        # per-k pieces on SP, in exact consumption order, so no big transfer
        # ever sits in the FIFO ahead of an earlier-needed piece
        t1 = stream.tile([128, 4, 512], F32R, tag="xt_c", name="xt_c1")
        stream_tiles[1] = t1
        W["wux"] = consts.tile([128, 4, HID], F32R, tag="w_wux", name="w_wux")
        W["wox"] = consts.tile([128, 4, HID], F32R, tag="w_wox", name="w_wox")
        W["wfx"] = consts.tile([128, 4, HID], F32R, tag="w_wfx", name="w_wfx")
        W["wfh"] = consts.tile([128, 4, HID], F32R, tag="w_wfh", name="w_wfh")
        xt3 = stream.tile([128, 4, 512], F32R, tag="xt_c", name="xt3")
        for k in range(4):
            nc.sync.dma_start(out=t1[:, k, :], in_=t_view(xt)[:, k, 512:1024].bitcast(F32R))
        for k in range(4):
            nc.sync.dma_start(out=W["wux"][:, k, :], in_=t_view(wps["wux"])[:, k, :].bitcast(F32R))
        for k in range(4):
            nc.sync.dma_start(out=xt3[:, k, :], in_=t_view(xt)[:, k, C3_OFF : C3_OFF + 512].bitcast(F32R))
        for k in range(4):
            nc.sync.dma_start(out=W["wfx"][:, k, :], in_=t_view(wps["wfx"])[:, k, :].bitcast(F32R))
        for k in range(4):
            nc.sync.dma_start(out=W["wfh"][:, k, :], in_=t_view(wps["wfh"])[:, k, :].bitcast(F32R))
        for k in range(4):
            nc.sync.dma_start(out=W["wox"][:, k, :], in_=t_view(wps["wox"])[:, k, :].bitcast(F32R))

        # persistent accumulators
        hsum3T = longp.tile([128, 4, 512], F32R, tag="hsum3T")
        fcsum3T = longp.tile([128, 4, 512], F32, tag="fcsum3T")
        fx3T = longp.tile([128, 4, 512], F32, tag="fx3T")

        def gate_T(g, rhs_x, rhs_h, first_k_outer=False):
            """Transposed-layout gate accumulation into 4 single-bank psum
            tiles (one per m) so each bank frees as soon as its m-tile is
            evacuated — PSUM occupancy, not engine time, is the scarce
            resource here (only 8 banks).
            pre[m][:,:] = sum_k WgxT[k,m].T @ rhs_x[k] (+ WghT h-part)"""
            ps = [psum.tile([128, 512], F32, tag="ps", name=f"ps{m}") for m in range(4)]

            if first_k_outer:
                # k-outer so each arriving (x, w) k-tile pair is consumed asap
                for k in range(4):
                    for m in range(4):
                        nc.tensor.matmul(
                            ps[m][:, :], W["w" + g + "x"][:, k, m * 128 : (m + 1) * 128],
                            rhs_x[:, k, :], start=(k == 0), stop=(k == 3),
                        )
                return ps
            for m in range(4):
                ms = slice(m * 128, (m + 1) * 128)
                for k in range(4):
                    nc.tensor.matmul(
                        ps[m][:, :], W["w" + g + "x"][:, k, ms], rhs_x[:, k, :],
                        start=(k == 0), stop=(rhs_h is None and k == 3),
                    )
            if rhs_h is not None:
                # all x-parts first: the h operand (hsum3T) finishes late, so
                # the 16 x matmuls buy PE cover for its arrival
                for m in range(4):
                    ms = slice(m * 128, (m + 1) * 128)
                    for k in range(4):
                        nc.tensor.matmul(
                            ps[m][:, :], W["w" + g + "h"][:, k, ms], rhs_h[:, k, :],
                            start=False, stop=(k == 3),
                        )
            return ps

        def evac(ps, act, bias_g, out_sb):
            for m in range(4):
                b = 0.0 if bias_g is None else bTs[bias_g][:, m : m + 1]
                nc.scalar.activation(out_sb[:, m, :], ps[m][:, :], act, bias=b)

        LOWP = "f32r rounding for downstream matmul"

        def fx3T_compute():
            # fx3T[hid, node] = (x3 @ WfxT) transposed + f bias, kept in the
            # transposed activation layout for the post-matmul broadcast add.
            psx = [psum.tile([128, 512], F32, tag="ps", name=f"ps{m}") for m in range(4)]
            for m in range(4):
                for k in range(4):
                    nc.tensor.matmul(
                        psx[m][:, :], W["wfx"][:, k, m * 128 : (m + 1) * 128],
                        xt3[:, k, :], start=(k == 0), stop=(k == 3),
                    )
            for m in range(4):
                nc.scalar.activation(fx3T[:, m, :], psx[m][:, :], ACTF.Copy)
            # fold the f bias in once (Pool, SBUF in-place) so the per-chunk
            # f sigmoids need no bias and can run as two merged ACT ops
            for m in range(4):
                nc.gpsimd.tensor_scalar_add(fx3T[:, m, :], fx3T[:, m, :],
                                            bTs["f"][:, m : m + 1])

        # ---------------- leaf phase ----------------
        # The f-gate matmuls for chunk c need C(c) (a DVE product of ACT
        # outputs); running them one chunk behind keeps PE from stalling on
        # the ACT/DVE tail of the current chunk.
        def leaf_fpath_mm(c, C_prev):
            # pre_f = Wfh @ C (normal psum group); then DVE adds the
            # broadcast fx3T(+bias) slice reading PSUM directly (per m, so
            # each bank frees in a pipelined wave).
            ps_f = [psum.tile([128, 512], F32, tag="ps", name=f"ps{m}") for m in range(4)]
            for m in range(4):
                ms = slice(m * 128, (m + 1) * 128)
                for k in range(4):
                    nc.tensor.matmul(
                        ps_f[m][:, :], W["wfh"][:, k, ms], C_prev[:, k, :],
                        start=(k == 0), stop=(k == 3),
                    )
            f_sb = workB.tile([128, 4, 512], F32, tag="Ug")
            for m in range(4):
                src = fx3T[:, m, 32 * c : 32 * c + 32]
                nc.vector.tensor_tensor(
                    out=f_sb[:, m, :].rearrange("p (n w) -> p n w", w=16),
                    in0=ps_f[m][:, :].rearrange("p (n w) -> p n w", w=16),
                    in1=src[:, :, None].broadcast_to([128, 32, 16]),
                    op=OP.add,
                )
            return f_sb

        def leaf_fpath_sigma(c, C_prev, f_sb, last=False):
            # sigma is emitted after the current chunk's tanh: its input (the
            # DVE add) lands late, and ACT's in-order queue must not block
            # the next chunk's evacuations behind it
            for h in range(2):
                nc.scalar.activation(f_sb[:, 2 * h : 2 * h + 2, :],
                                     f_sb[:, 2 * h : 2 * h + 2, :], ACTF.Sigmoid)
            fC_sb = workB.tile([128, 4, 512], F32, tag="H")
            # last chunk: fC on DVE (faster than Pool) so fcsum3T completes
            # before the level-3 out-DMAs queue up behind fc3T's transfer
            eng = nc.vector if last else nc.gpsimd
            eng.tensor_mul(fC_sb[:, :, :], f_sb[:, :, :], C_prev[:, :, :].bitcast(F32))
            nc.vector.tensor_reduce(
                fcsum3T[:, :, 32 * c : 32 * c + 32],
                fC_sb[:, :, :].rearrange("p t (g w) -> p t g w", w=16),
                axis=AX.X, op=OP.add,
            )

        def leaf_hpath(c, C_cur, o_cur):
            tC_sb = workA.tile([128, 4, 512], F32, tag="T")
            H_sb = workB.tile([128, 4, 512], F32, tag="H")
            nc.scalar.activation(tC_sb[:, :, :], C_cur[:, :, :].bitcast(F32), ACTF.Tanh)
            nc.vector.tensor_mul(H_sb[:, :, :], o_cur[:, :, :], tC_sb[:, :, :])
            with nc.allow_low_precision(LOWP):
                nc.vector.tensor_reduce(
                    hsum3T[:, :, 32 * c : 32 * c + 32],
                    H_sb[:, :, :].rearrange("p t (g w) -> p t g w", w=16),
                    axis=AX.X, op=OP.add,
                )

        # Paired c0/c1 warmup: six gate waves in a row give the serial DMA
        # FIFO ~22us of PE cover to stream x(c0), wix, x(c1), wux, wox
        # before each is first consumed.
        def gate(g, xt_c, act, bias_g, pool, tag, first_k_outer=False):
            ps = gate_T(g, xt_c, None, first_k_outer=first_k_outer)
            sb = pool.tile([128, 4, 512], F32, tag=tag)
            evac(ps, act, bias_g, sb)
            return sb

        xt_c1 = stream_tiles[1]
        i_sb0 = gate("i", t0, ACTF.Sigmoid, "i", workA, "A", first_k_outer=True)
        i_sb1 = gate("i", xt_c1, ACTF.Sigmoid, "i", workA, "A")
        u_sb0 = gate("u", t0, ACTF.Tanh, "u", workB, "Ug")
        u_sb1 = gate("u", xt_c1, ACTF.Tanh, "u", workB, "Ug")
        C_sb0 = workA.tile([128, 4, 512], F32R, tag="C")
        nc.vector.tensor_mul(C_sb0[:, :, :], i_sb0[:, :, :], u_sb0[:, :, :])
        fx3T_compute()
        f_sb0 = leaf_fpath_mm(0, C_sb0)
        o_sb0 = gate("o", t0, ACTF.Sigmoid, "o", workB, "B")
        o_sb1 = gate("o", xt_c1, ACTF.Sigmoid, "o", workB, "B")
        C_sb1 = workA.tile([128, 4, 512], F32R, tag="C")
        nc.vector.tensor_mul(C_sb1[:, :, :], i_sb1[:, :, :], u_sb1[:, :, :])

        leaf_hpath(0, C_sb0, o_sb0)
        leaf_fpath_sigma(0, C_sb0, f_sb0)
        leaf_hpath(1, C_sb1, o_sb1)

        pipe = (1, C_sb1)
        for c in range(2, N_CHUNK):
            xt_c = load_chunk(c)
            if c >= 12 and c <= 14:
                # late h-part weights, one per iteration: the queue is idle
                # here and they ride spare stream-pool slots until level 3
                n = ("wih", "woh", "wuh")[c - 12]
                W[n] = stream.tile([128, 4, HID], F32R, tag="xt_c", name="w_" + n)
                nc.gpsimd.dma_start(out=W[n][:, :, :], in_=t_view(wps[n]).bitcast(F32R))
            if c == 15:
                # xt3's warmup slot was recycled after fx3T; reload for L3
                xt3_2 = stream.tile([128, 4, 512], F32R, tag="xt_c", name="xt3_2")
                nc.sync.dma_start(
                    out=xt3_2[:, :, :],
                    in_=t_view(xt)[:, :, C3_OFF : C3_OFF + 512].bitcast(F32R),
                )

            i_sb = gate("i", xt_c, ACTF.Sigmoid, "i", workA, "A")
            u_sb = gate("u", xt_c, ACTF.Tanh, "u", workB, "Ug")
            if c == 15:
                # last chunk: C15 -> tanh -> H -> hsum15 is the critical
                # chain into the level-3 h-parts; emit it at queue heads
                C_sb = workA.tile([128, 4, 512], F32R, tag="C")
                nc.vector.tensor_mul(C_sb[:, :, :], i_sb[:, :, :], u_sb[:, :, :])
                f_sb = leaf_fpath_mm(pipe[0], pipe[1])
                o_sb = gate("o", xt_c, ACTF.Sigmoid, "o", workB, "B")
                leaf_hpath(c, C_sb, o_sb)
                leaf_fpath_sigma(pipe[0], pipe[1], f_sb)
            else:
                f_sb = leaf_fpath_mm(pipe[0], pipe[1])
                o_sb = gate("o", xt_c, ACTF.Sigmoid, "o", workB, "B")

                C_sb = workA.tile([128, 4, 512], F32R, tag="C")
                nc.vector.tensor_mul(C_sb[:, :, :], i_sb[:, :, :], u_sb[:, :, :])

                leaf_hpath(c, C_sb, o_sb)
                leaf_fpath_sigma(pipe[0], pipe[1], f_sb)
            pipe = (c, C_sb)

        f_sb = leaf_fpath_mm(pipe[0], pipe[1])
        leaf_fpath_sigma(pipe[0], pipe[1], f_sb, last=True)

        # ---------------- level 3 (512 nodes, transposed) ----------------
        # fcsum3T's DMA is emitted first so its transfer leads the out FIFO.
        nc.sync.dma_start(out=out_t["fc3T"][:, :, :], in_=fcsum3T[:, :, :])

        def l3_gate(g, act, out_name, sb_pool, sb_tag):
            ps = gate_T(g, xt3_2, hsum3T)
            sb = sb_pool.tile([128, 4, 512], F32, tag=sb_tag)
            for m in range(4):
                nc.scalar.activation(sb[:, m, :], ps[m][:, :], act,
                                     bias=bTs[g][:, m : m + 1])
                nc.sync.dma_start(out=out_t[out_name][:, m, :], in_=sb[:, m, :])
            return sb

        l3_gate("i", ACTF.Sigmoid, "i3T", workA, "A")
        l3_gate("u", ACTF.Tanh, "u3T", workB, "Ug")
        l3_gate("o", ACTF.Sigmoid, "o3T", workB, "B")

    nc.finalize()
    return nc


def _np_sigmoid(v):
    return 1.0 / (1.0 + np.exp(-v))


def _host_prep(x, wi_w, wo_w, wu_w, wf_w, wi_b, wo_b, wu_b, wf_b):
    xt_full = np.ascontiguousarray(x.T)  # [512, 69905]

    def wT(w, part):
        return np.ascontiguousarray(w[:, :512].T if part == "x" else w[:, 512:].T)

    common = {
        "wix": wT(wi_w, "x"), "wih": wT(wi_w, "h"),
        "wox": wT(wo_w, "x"), "woh": wT(wo_w, "h"),
        "wux": wT(wu_w, "x"), "wuh": wT(wu_w, "h"),
        "wfx": wT(wf_w, "x"), "wfh": wT(wf_w, "h"),
        "bTi": np.ascontiguousarray(np.asarray(wi_b).reshape(4, 128).T),
        "bTo": np.ascontiguousarray(np.asarray(wo_b).reshape(4, 128).T),
        "bTu": np.ascontiguousarray(np.asarray(wu_b).reshape(4, 128).T),
        "bTf": np.ascontiguousarray(np.asarray(wf_b).reshape(4, 128).T),
    }
    in_maps = []
    for c in range(NC_N):
        xt_c = np.concatenate(
            [
                xt_full[:, OFFS[4] + 8192 * c : OFFS[4] + 8192 * (c + 1)],
                xt_full[:, OFFS[3] + 512 * c : OFFS[3] + 512 * (c + 1)],
            ],
            axis=1,
        )
        in_maps.append({"xt": np.ascontiguousarray(xt_c), **common})
    return in_maps


def _t_to_nodes(a):
    """[128, 4, n] transposed tile -> [n, 512] natural (hid = t*128 + p)."""
    return np.ascontiguousarray(np.transpose(np.asarray(a), (2, 1, 0)).reshape(a.shape[2], 512))


def _host_finish(x, res, wi_w, wi_b, wf_w, wf_b, wo_w, wo_b, wu_w, wu_b):
    """Levels 2..0 in float64 from per-core level-3 gate outputs."""
    f8 = np.float64
    i3 = np.concatenate([_t_to_nodes(res[c]["i3T"]) for c in range(NC_N)]).astype(f8)
    u3 = np.concatenate([_t_to_nodes(res[c]["u3T"]) for c in range(NC_N)]).astype(f8)
    o3 = np.concatenate([_t_to_nodes(res[c]["o3T"]) for c in range(NC_N)]).astype(f8)
    fc3 = np.concatenate([_t_to_nodes(res[c]["fc3T"]) for c in range(NC_N)]).astype(f8)

    C = i3 * u3 + fc3  # [4096, 512]
    H = o3 * np.tanh(C)

    wi = np.asarray(wi_w, f8)
    wo = np.asarray(wo_w, f8)
    wu = np.asarray(wu_w, f8)
    wf = np.asarray(wf_w, f8)
    bi, bo, bu, bf = (np.asarray(b, f8) for b in (wi_b, wo_b, wu_b, wf_b))

    offs = OFFS
    for d in range(2, -1, -1):
        Xd = np.asarray(x[offs[d] : offs[d + 1]], f8)  # [n, 512]
        n = Xd.shape[0]
        Hc = H.reshape(n, BRANCH, HID)
        Cc = C.reshape(n, BRANCH, HID)
        h_sum = Hc.sum(axis=1)
        xh = np.concatenate([Xd, h_sum], axis=1)
        i = _np_sigmoid(xh @ wi.T + bi)
        o = _np_sigmoid(xh @ wo.T + bo)
        u = np.tanh(xh @ wu.T + bu)
        fx = Xd @ wf[:, :IN].T  # [n, 512]
        fc = (C @ wf[:, IN:].T).reshape(n, BRANCH, HID)
        f = _np_sigmoid(fc + fx[:, None, :] + bf)
        C = i * u + (f * Cc).sum(axis=1)
        H = o * np.tanh(C)

    return H[0].astype(np.float32), C[0].astype(np.float32)


def _run(in_maps, trace=False):
    from concourse.bass_utils import run_bass_kernel_spmd

    if "nc" not in _CACHE:
        _CACHE["nc"] = _build_nc()
    return run_bass_kernel_spmd(_CACHE["nc"], in_maps, list(range(NC_N)), trace=trace)


def kernel(x, wi_w, wi_b, wf_w, wf_b, wo_w, wo_b, wu_w, wu_b, _trace=False):
    x = np.asarray(x, np.float32)
    in_maps = _host_prep(x, wi_w, wo_w, wu_w, wf_w, wi_b, wo_b, wu_b, wf_b)
    res = _run(in_maps, trace=_trace)
    _CACHE["last_results"] = res
    H0, C0 = _host_finish(x, res.results, wi_w, wi_b, wf_w, wf_b, wo_w, wo_b, wu_w, wu_b)
    return H0, C0
